# revision 61
# baseline (speedup 1.0000x reference)
"""Trainium2 Bass kernel for causal multi-head attention.

Problem: nn_MultiHeadAttention (B=4, N=2048, D=768, H=12, dh=64), fp32 I/O.

Sharding: 8 cores = 4 batches x 2 head-groups (6 heads each).  Each core
computes QKV projections for its 6 heads, causal softmax attention, and a
partial output projection (its heads' rows of Wo).  The two partials per
batch are summed on the host (tensor-parallel reduce); the bias is added on
the hg=0 core only.

Per-core layout strategy (all matmuls in bf16, fp32 accumulate):
  - X^T is prepared host-side: xt[c,p,n] = X[n, 128c+p] (bf16).
  - Q^T, K^T for pair 0 computed kc-outer (4 PSUM accumulators) so the PE
    starts as soon as the first xt/wq chunks land; later pairs' projections
    stream as fill work inside the attention loop.
  - V computed in natural [N, 64h] layout, extended with a ones column per
    head so the context matmul also produces the softmax denominators.
  - scores^T tiles [k=128, 2 heads, q=512] in PSUM, exp on ScalarE
    (scale=1/8 fused), causal diag masked by triangle multiply.
  - ctx^T accumulated in PSUM over k chunks; row 64 = sum_k exp (denom).
  - normalization is DEFERRED: each block's ctx+denominators are copied to
    SBUF and a "finisher" generator is queued as fill work, so the serial
    chain (denominator row -> partition 0 via DMA -> gpsimd
    partition_broadcast -> DVE reciprocal_approx_fast -> per-q scale;
    head 1 moves to partitions 64-127 via an SBUF->SBUF DMA) never blocks
    the in-order PE queue.
  - out = ctxn^T.T @ Wo + bias (two e-chunks per PSUM tile so the c=0/1
    contractions pre-run and only c=2 waits on the pair-2 normalization),
    bias fused into the PSUM->SBUF copy.
"""

import sys

sys.path.insert(0, "/opt/trn_rl_repo")

import numpy as np
import ml_dtypes

BF16 = ml_dtypes.bfloat16

P = 128
DIN = 768
DH = 384  # per-core output cols of Wq/Wk/Wv (6 heads x 64)
NH = 6  # heads per core
KCH = 6  # d_in chunks (768/128)
QW = 512  # q block width


def build(seq=2048, n_wchunks=3):
    """Build the SPMD single-core program.  seq parameterized for sim tests."""
    import concourse.mybir as mybir
    import concourse.tile as tile
    from concourse import bacc
    from contextlib import ExitStack

    f32 = mybir.dt.float32
    f32r = mybir.dt.float32r
    bf16 = mybir.dt.bfloat16
    EXP = mybir.ActivationFunctionType.Exp

    nqb = seq // QW  # q blocks of 512
    nkc = seq // P  # k chunks of 128
    HP = 3  # head pairs

    nc = bacc.Bacc(None, target_bir_lowering=False, debug=False)

    xt_d = nc.dram_tensor("xt", [KCH, P, seq], bf16, kind="ExternalInput")
    wq_d = nc.dram_tensor("wq", [KCH, P, DH], bf16, kind="ExternalInput")
    wk_d = nc.dram_tensor("wk", [KCH, P, DH], bf16, kind="ExternalInput")
    wv_d = nc.dram_tensor("wv", [KCH, P, DH], bf16, kind="ExternalInput")
    wo_d = nc.dram_tensor("wo", [n_wchunks, P, DIN], bf16, kind="ExternalInput")
    # Wo rows of the last head pair's h1 (chunk 2 rows 64-127), staged at
    # partitions 0-63 so the tail's c2b matmul can read tt directly
    wo2b_d = nc.dram_tensor("wo2b", [64, DIN], bf16, kind="ExternalInput")
    bias_d = nc.dram_tensor("bias", [P, DIN // P], f32, kind="ExternalInput")
    tri_d = nc.dram_tensor("tri", [P, P], bf16, kind="ExternalInput")
    # output is stored transposed: out[e_chunk, e_p, q] = full_out[q, 128*e_chunk+e_p]
    out_d = nc.dram_tensor("out", [DIN // P, P, seq], f32, kind="ExternalOutput")

    with tile.TileContext(nc) as tc, ExitStack() as ctx:
        const = ctx.enter_context(tc.tile_pool(name="const", bufs=1))
        io = ctx.enter_context(tc.tile_pool(name="io", bufs=1))
        expp = ctx.enter_context(tc.tile_pool(name="expp", bufs=8))
        crawp = ctx.enter_context(tc.tile_pool(name="crawp", bufs=4))
        smallp = ctx.enter_context(tc.tile_pool(name="smallp", bufs=4))
        invbp = ctx.enter_context(tc.tile_pool(name="invbp", bufs=2))
        outp = ctx.enter_context(tc.tile_pool(name="outp", bufs=3))
        ps = ctx.enter_context(tc.tile_pool(name="ps", bufs=3, space="PSUM"))
        cxps = ctx.enter_context(tc.tile_pool(name="cxps", bufs=1, space="PSUM"))

        # ---------------- persistent inputs ----------------
        xt = const.tile([P, KCH, seq], bf16, name="xt_sb")
        wq = const.tile([P, KCH, DH], bf16, name="wq_sb")
        wk = const.tile([P, KCH, DH], bf16, name="wk_sb")
        wv = const.tile([P, KCH, DH], bf16, name="wv_sb")
        wo = const.tile([P, n_wchunks, DIN], bf16, name="wo_sb")
        wo2b = const.tile([64, DIN], bf16, name="wo2b_sb")
        bias = const.tile([P, DIN // P], f32, name="bias_sb")
        tri = const.tile([P, P], bf16, name="tri_sb")

        # DMA order: per d_in chunk, xt on the sync queue and wq/wk on the
        # scalar queue, so the kc-outer first projections can start as soon
        # as chunk 0 lands.
        for c in range(KCH):
            if c == 0:
                # split the first chunk so the first projection matmul can
                # start as soon as its 512-col slice lands
                q4 = seq // 4
                for qtr in range(4):
                    qs = slice(qtr * q4, (qtr + 1) * q4)
                    nc.sync.dma_start(xt[:, 0, qs], xt_d[0, :, qs])
            else:
                nc.sync.dma_start(xt[:, c, :], xt_d[c])
            nc.scalar.dma_start(wq[:, c, :], wq_d[c])
            nc.scalar.dma_start(wk[:, c, :], wk_d[c])
            nc.scalar.dma_start(wv[:, c, :], wv_d[c])
        nc.scalar.dma_start(tri[:], tri_d[:])
        for c in range(n_wchunks):
            nc.scalar.dma_start(wo[:, c, :], wo_d[c])
        nc.scalar.dma_start(wo2b[:], wo2b_d[:])
        nc.scalar.dma_start(bias[:], bias_d[:])

        # persistent activations
        qt = io.tile([P, HP, seq], bf16, name="qt_sb")
        ttl = io.tile([64, QW], bf16, name="ttl_sb")  # tail block's h1 ctx
        kt = io.tile([P, HP, seq], bf16, name="kt_sb")
        vx = io.tile([P, nkc, NH, 65], bf16, name="vx_sb")
        cn = io.tile([P, HP, seq], bf16, name="cn_sb")
        nc.vector.memset(vx[:, :, :, 64:65], 1.0)

        def qk0_upfront():
            """Pair-0 Q^T and K^T, kc-outer over 4 PSUM accumulators so each
            matmul only needs xt chunk kc (overlaps the input DMA stream)."""
            specs = []
            for wt, dst in ((wq, qt), (wk, kt)):
                for nb0 in (0, 2):
                    nbs = tuple(nb for nb in (nb0, nb0 + 1) if nb < nqb)
                    if nbs:
                        specs.append((wt, dst, nbs))
            accs = []
            for si in range(len(specs)):
                if si == len(specs) - 1:
                    acc = cxps.tile([P, 2, QW], f32, tag="cx", name=f"qk0acc{si}")
                else:
                    acc = ps.tile([P, 2, QW], f32, tag="quad", name=f"qk0acc{si}")
                accs.append(acc)
            for kc in range(KCH):
                for acc, (wt, dst, nbs) in zip(accs, specs):
                    for r, nb in enumerate(nbs):
                        nc.tensor.matmul(
                            acc[:, r, :],
                            lhsT=wt[:, kc, 0:P],
                            rhs=xt[:, kc, nb * QW : (nb + 1) * QW],
                            start=(kc == 0),
                            stop=(kc == KCH - 1),
                        )
            for acc, (wt, dst, nbs) in zip(accs, specs):
                for r, nb in enumerate(nbs):
                    nc.vector.tensor_copy(dst[:, 0, nb * QW : (nb + 1) * QW], acc[:, r, :])

        def qk_quarter(pair, quarter):
            """Project one quarter of pair's Q^T/K^T: one weight chunk reused
            across two 512-wide n blocks (kc-outer keeps LDWEIGHTS warm).
            Yields after each matmul so the caller can interleave."""
            wt, dst = (wq, qt) if quarter < 2 else (wk, kt)
            nbs = (0, 1) if quarter % 2 == 0 else (2, 3)
            if nbs[-1] >= nqb:  # small-seq (sim) builds
                nbs = tuple(nb for nb in nbs if nb < nqb)
                if not nbs:
                    return
            pt = ps.tile([P, 2, QW], f32, tag="quad", name="pt")
            for kc in range(KCH):
                for r, nb in enumerate(nbs):
                    nc.tensor.matmul(
                        pt[:, r, :],
                        lhsT=wt[:, kc, pair * P : (pair + 1) * P],
                        rhs=xt[:, kc, nb * QW : (nb + 1) * QW],
                        start=(kc == 0),
                        stop=(kc == KCH - 1),
                    )
                    yield
            for r, nb in enumerate(nbs):
                nc.vector.tensor_copy(dst[:, pair, nb * QW : (nb + 1) * QW], pt[:, r, :])

        def v_chunk(nb):
            """Yields after each matmul so the caller can interleave."""
            pt = ps.tile([P, 2, QW], f32, tag="quad", name="pt")
            for kc in range(KCH):
                nc.tensor.matmul(
                    pt[:, 0, :DH],
                    lhsT=xt[:, kc, nb * P : (nb + 1) * P],
                    rhs=wv[:, kc, :],
                    start=(kc == 0),
                    stop=(kc == KCH - 1),
                )
                yield
            nc.vector.tensor_copy(
                vx[:, nb, :, 0:64],
                pt[:, 0, :DH].rearrange("p (h d) -> p h d", d=64),
            )

        def out_proj_t(e0, qb, split_c2=False):
            """Transposed output projection for TWO e-chunks (both halves of
            one PSUM tile): out^T[e-chunk, q-block] = Wo_chunk^T @ cn, bias
            as a per-partition scalar.  c=0/1 for both e-chunks run first so
            only the c=2 matmuls (this core's last head pair) wait on the
            pair-2 normalization.  Yields after each matmul."""
            qsl = slice(qb * QW, (qb + 1) * QW)
            op = ps.tile([P, 2, QW], f32, tag="quad", name="op")
            es = [e for e in (e0, e0 + 1) if e < DIN // P]
            for c in range(n_wchunks - 1):
                for r, e in enumerate(es):
                    nc.tensor.matmul(
                        op[:, r, :],
                        lhsT=wo[:, c, e * P : (e + 1) * P],
                        rhs=cn[:, c, qsl],
                        start=(c == 0),
                        stop=False,
                    )
                    yield
            c2 = n_wchunks - 1
            for r, e in enumerate(es):
                if split_c2:
                    nc.tensor.matmul(
                        op[:, r, :],
                        lhsT=wo[0:64, c2, e * P : (e + 1) * P],
                        rhs=cn[0:64, c2, qsl],
                        start=False,
                        stop=False,
                    )
                    yield
                    nc.tensor.matmul(
                        op[:, r, :],
                        lhsT=wo2b[:, e * P : (e + 1) * P],
                        rhs=ttl[:],
                        start=False,
                        stop=True,
                    )
                else:
                    nc.tensor.matmul(
                        op[:, r, :],
                        lhsT=wo[:, c2, e * P : (e + 1) * P],
                        rhs=cn[:, c2, qsl],
                        start=False,
                        stop=True,
                    )
                yield
            for r, e in enumerate(es):
                ob = outp.tile([P, QW], f32, name="ob")
                nc.vector.tensor_scalar_add(ob[:], op[:, r, :], bias[:, e : e + 1])
                nc.sync.dma_start(out_d[e, :, qsl], ob[:])

        def finisher(pair, i, cr, tail=0, last=False):
            """Deferred per-block softmax normalization, run as fill work.
            The denominator row lives on partition 64 and engines can't
            cross partitions, so: DMA-scatter it across 128 lanes,
            reciprocal, DRAM round-trip partition-broadcast, per-q scale on
            DVE, head-1 partition move via SBUF->SBUF DMA.  The yields pace
            each op's queue position to just after its data dependency;
            `tail` delays whatever is queued next (out_proj) past the DMA
            chain's latency."""
            qsl = slice(i * QW, (i + 1) * QW)
            # move the denominator row to partition 0 (engines can't cross
            # partitions; gpsimd's broadcast only reads partition 0)
            dn = smallp.tile([1, 2, QW], f32, tag="dn", bufs=2, name="dn")
            nc.sync.dma_start(dn[:], cr[64:65, :, :])
            yield
            yield
            db = invbp.tile([64, 2, QW], f32, tag="db", name="db")
            nc.gpsimd.partition_broadcast(db[:], dn[0:1, :, :])
            yield
            ib = invbp.tile([64, 2, QW], f32, tag="ib", name="ib")
            nc.vector.reciprocal_approx_fast(ib[:], db[:])
            yield
            nc.vector.tensor_mul(cn[0:64, pair, qsl], cr[0:64, 0, :], ib[:, 0, :])
            yield
            if last:
                # h1 stays in ttl (partitions 0-63); the split c2b matmul
                # reads it directly, skipping the partition-move DMA
                nc.vector.tensor_mul(ttl[:], cr[0:64, 1, :], ib[:, 1, :])
            else:
                tt = smallp.tile([64, QW], bf16, tag="tt", name="tt")
                nc.vector.tensor_mul(tt[:], cr[0:64, 1, :], ib[:, 1, :])
                yield
                nc.sync.dma_start(cn[64:128, pair, qsl], tt[:])
            for _ in range(tail):
                yield

        class FillQueue:
            """Queue of instruction generators, driven a few matmuls at a
            time from inside the attention loop to fill PE bubbles.
            Finishers are urgent and small: they jump the queue, but never
            interrupt a generator that has already started (its PSUM
            accumulator is live)."""

            def __init__(self):
                self.gens = []  # (label, gen)
                self.head_started = False

            def add(self, gen, label=None):
                self.gens.append((label, gen))

            def add_front(self, gen, label=None):
                pos = 1 if (self.head_started and self.gens) else 0
                self.gens.insert(pos, (label, gen))

            def _advance(self):
                """One step of the head generator. Returns False if empty."""
                if not self.gens:
                    return False
                try:
                    next(self.gens[0][1])
                    self.head_started = True
                except StopIteration:
                    self.gens.pop(0)
                    self.head_started = False
                return True

            def step(self, n):
                while n > 0 and self.gens:
                    self._advance()
                    n -= 1

            def drain_matching(self, label):
                """Run all generators with this label to completion, in order,
                leaving other generators queued (and positioned)."""
                k = 0
                while k < len(self.gens):
                    if self.gens[k][0] == label:
                        gen = self.gens[k][1]
                        while True:
                            try:
                                next(gen)
                            except StopIteration:
                                break
                        self.gens.pop(k)
                        if k == 0:
                            self.head_started = False
                    else:
                        k += 1

            def count(self, label):
                return sum(1 for lab, _ in self.gens if lab == label)

            def drain(self):
                while self.gens:
                    self._advance()

        def attention_qblock(pair, i, fq, steps=(2, 3)):
            """One 512-wide q block of causal attention for a head pair.
            fq: FillQueue driven mid-loop to fill PE bubbles."""
            nj = 4 * i + 4  # active k chunks
            cx = cxps.tile([65, 2, QW], f32, tag="cx", name="cx")
            pend = []  # (j, exp tile) awaiting ctx matmuls

            def ctx_mms(j, et):
                d = j - 4 * i
                off = P * d if d > 0 else 0
                for hh in (0, 1):
                    nc.tensor.matmul(
                        cx[:, hh, off:],
                        lhsT=vx[:, j, 2 * pair + hh, :],
                        rhs=et[:, hh, off:],
                        start=(j == 0),
                        stop=(j == nj - 1),
                    )

            for j in range(nj):
                d = j - 4 * i
                off = P * d if d > 0 else 0
                sc = ps.tile([P, 2, QW], f32, tag="quad", name="sc")
                for hh in (0, 1):
                    base = 64 * hh
                    nc.tensor.matmul(
                        sc[:, hh, off:],
                        lhsT=kt[base : base + 64, pair, j * P : (j + 1) * P],
                        rhs=qt[base : base + 64, pair, i * QW + off : (i + 1) * QW],
                        start=True,
                        stop=True,
                        tile_position=(base, 0),
                    )
                et = expp.tile([P, 2, QW], bf16, name="et")
                if off:
                    nc.scalar.activation(
                        et[:, :, off:], sc[:, :, off:], EXP, scale=0.125
                    )
                else:
                    nc.scalar.activation(et[:], sc[:], EXP, scale=0.125)
                if d >= 0:
                    for hh in (0, 1):
                        nc.vector.tensor_mul(
                            et[:, hh, off : off + P], et[:, hh, off : off + P], tri[:]
                        )
                if len(pend) >= 2:
                    ctx_mms(*pend.pop(0))
                pend.append((j, et))
                fq.step(steps[1] if d >= 0 else steps[0])
            for args in pend:
                ctx_mms(*args)

            # ctx + denominators out of PSUM; normalization deferred to a
            # finisher so the next block's matmuls proceed immediately.
            # Safety valve: bound pending finishers (each pins a cr buffer)
            # by draining from the front, preserving FIFO start order.
            while fq.count("fin") > 2:
                fq._advance()
            cr = crawp.tile([65, 2, QW], f32, name="cr")
            if pair == HP - 1 and i == nqb - 1:
                # very last block: copy the denominator row first so the
                # finisher's DMA chain starts ~0.5us earlier
                nc.vector.tensor_copy(cr[64:65, :, :], cx[64:65, :, :])
                nc.vector.tensor_copy(cr[0:64, :, :], cx[0:64, :, :])
            else:
                nc.vector.tensor_copy(cr[:], cx[:])
            fq.add_front(
                finisher(
                    pair,
                    i,
                    cr,
                    tail=8 if pair == HP - 1 else 0,
                    last=False,
                ),
                "fin",
            )

        # ---------------- schedule ----------------
        BISECT_QK0 = False
        if BISECT_QK0:
            for quarter in range(4):
                for _ in qk_quarter(0, quarter):
                    pass
        else:
            qk0_upfront()
        for nb in range(nkc):
            for _ in v_chunk(nb):
                pass
        fq = FillQueue()
        for pair in range(HP):
            last_pair = pair == HP - 1
            if not last_pair:
                for quarter in range(4):
                    fq.add(qk_quarter(pair + 1, quarter), f"qk{pair + 1}")
            for i in range(nqb):
                attention_qblock(pair, i, fq, steps=(3, 4))
                if last_pair:
                    for e0 in range(0, DIN // P, 2):
                        fq.add(out_proj_t(e0, i, split_c2=False), "op")
            if not last_pair:
                fq.drain_matching(f"qk{pair + 1}")
        fq.drain()

    nc.compile()
    return nc


def make_in_maps(X, Wq, Wk, Wv, Wo, bo, seq=2048):
    """Shard full inputs into the 8 per-core input maps."""
    X = np.asarray(X, np.float32)
    Wq = np.asarray(Wq, np.float32)
    Wk = np.asarray(Wk, np.float32)
    Wv = np.asarray(Wv, np.float32)
    Wo = np.asarray(Wo, np.float32)
    bo = np.asarray(bo, np.float32)

    tri = np.triu(np.ones((P, P), np.float32)).astype(BF16)
    bias_full = np.ascontiguousarray(bo.reshape(DIN // P, P).T).astype(np.float32)
    bias_zero = np.zeros((P, DIN // P), np.float32)

    in_maps = []
    for b in range(X.shape[0]):
        xt = np.ascontiguousarray(X[b].T).astype(BF16).reshape(KCH, P, seq)
        for hg in range(2):
            sl = slice(hg * DH, (hg + 1) * DH)
            in_maps.append(
                {
                    "xt": xt,
                    "wq": np.ascontiguousarray(Wq[:, sl]).astype(BF16).reshape(KCH, P, DH),
                    "wk": np.ascontiguousarray(Wk[:, sl]).astype(BF16).reshape(KCH, P, DH),
                    "wv": np.ascontiguousarray(Wv[:, sl]).astype(BF16).reshape(KCH, P, DH),
                    "wo": np.ascontiguousarray(Wo[sl, :]).astype(BF16).reshape(3, P, DIN),
                    "wo2b": np.ascontiguousarray(Wo[sl, :][320:384, :]).astype(BF16),
                    "bias": bias_full if hg == 0 else bias_zero,
                    "tri": tri,
                }
            )
    return in_maps


_built = None


def _get_built():
    global _built
    if _built is None:
        _built = build()
    return _built


def run(inputs, trace=False):
    from concourse.bass_utils import run_bass_kernel_spmd

    nc = _get_built()
    in_maps = make_in_maps(**inputs)
    res = run_bass_kernel_spmd(nc, in_maps, list(range(8)), trace=trace)
    # per-core output is stored transposed as [6, 128, seq] = out.T chunked
    parts = [
        np.asarray(r["out"], np.float32).reshape(DIN, -1).T for r in res.results
    ]
    out = np.stack([parts[2 * b] + parts[2 * b + 1] for b in range(len(parts) // 2)])
    return out, res


def kernel(X, Wq, Wk, Wv, Wo, bo):
    out, _ = run(dict(X=X, Wq=Wq, Wk=Wk, Wv=Wv, Wo=Wo, bo=bo))
    return out


# revision 62
# speedup vs baseline: 1.0137x; 1.0137x over previous
"""Trainium2 Bass kernel for causal multi-head attention.

Problem: nn_MultiHeadAttention (B=4, N=2048, D=768, H=12, dh=64), fp32 I/O.

Sharding: 8 cores = 4 batches x 2 head-groups (6 heads each).  Each core
computes QKV projections for its 6 heads, causal softmax attention, and a
partial output projection (its heads' rows of Wo).  The two partials per
batch are summed on the host (tensor-parallel reduce); the bias is added on
the hg=0 core only.

Per-core layout strategy (all matmuls in bf16, fp32 accumulate):
  - X^T is prepared host-side: xt[c,p,n] = X[n, 128c+p] (bf16).
  - Q^T, K^T for pair 0 computed kc-outer (4 PSUM accumulators) so the PE
    starts as soon as the first xt/wq chunks land; later pairs' projections
    stream as fill work inside the attention loop.
  - V computed in natural [N, 64h] layout, extended with a ones column per
    head so the context matmul also produces the softmax denominators.
  - scores^T tiles [k=128, 2 heads, q=512] in PSUM, exp on ScalarE
    (scale=1/8 fused), causal diag masked by triangle multiply.
  - ctx^T accumulated in PSUM over k chunks; row 64 = sum_k exp (denom).
  - normalization is DEFERRED: each block's ctx+denominators are copied to
    SBUF and a "finisher" generator is queued as fill work, so the serial
    chain (denominator row -> partition 0 via DMA -> gpsimd
    partition_broadcast -> DVE reciprocal_approx_fast -> per-q scale;
    head 1 moves to partitions 64-127 via an SBUF->SBUF DMA) never blocks
    the in-order PE queue.
  - out = ctxn^T.T @ Wo + bias (two e-chunks per PSUM tile so the c=0/1
    contractions pre-run and only c=2 waits on the pair-2 normalization),
    bias fused into the PSUM->SBUF copy.
"""

import sys

sys.path.insert(0, "/opt/trn_rl_repo")

import numpy as np
import ml_dtypes

BF16 = ml_dtypes.bfloat16

P = 128
DIN = 768
DH = 384  # per-core output cols of Wq/Wk/Wv (6 heads x 64)
NH = 6  # heads per core
KCH = 6  # d_in chunks (768/128)
QW = 512  # q block width


def build(seq=2048, n_wchunks=3):
    """Build the SPMD single-core program.  seq parameterized for sim tests."""
    import concourse.mybir as mybir
    import concourse.tile as tile
    from concourse import bacc
    from contextlib import ExitStack

    f32 = mybir.dt.float32
    f32r = mybir.dt.float32r
    bf16 = mybir.dt.bfloat16
    EXP = mybir.ActivationFunctionType.Exp

    nqb = seq // QW  # q blocks of 512
    nkc = seq // P  # k chunks of 128
    HP = 3  # head pairs

    nc = bacc.Bacc(None, target_bir_lowering=False, debug=False)

    xt_d = nc.dram_tensor("xt", [KCH, P, seq], bf16, kind="ExternalInput")
    wq_d = nc.dram_tensor("wq", [KCH, P, DH], bf16, kind="ExternalInput")
    wk_d = nc.dram_tensor("wk", [KCH, P, DH], bf16, kind="ExternalInput")
    wv_d = nc.dram_tensor("wv", [KCH, P, DH], bf16, kind="ExternalInput")
    wo_d = nc.dram_tensor("wo", [n_wchunks, P, DIN], bf16, kind="ExternalInput")
    # Wo rows of the last head pair's h1 (chunk 2 rows 64-127), staged at
    # partitions 0-63 so the tail's c2b matmul can read tt directly
    wo2b_d = nc.dram_tensor("wo2b", [64, DIN], bf16, kind="ExternalInput")
    bias_d = nc.dram_tensor("bias", [P, DIN // P], f32, kind="ExternalInput")
    tri_d = nc.dram_tensor("tri", [P, P], bf16, kind="ExternalInput")
    # output is stored transposed: out[e_chunk, e_p, q] = full_out[q, 128*e_chunk+e_p]
    out_d = nc.dram_tensor("out", [DIN // P, P, seq], f32, kind="ExternalOutput")

    with tile.TileContext(nc) as tc, ExitStack() as ctx:
        const = ctx.enter_context(tc.tile_pool(name="const", bufs=1))
        io = ctx.enter_context(tc.tile_pool(name="io", bufs=1))
        expp = ctx.enter_context(tc.tile_pool(name="expp", bufs=8))
        crawp = ctx.enter_context(tc.tile_pool(name="crawp", bufs=4))
        smallp = ctx.enter_context(tc.tile_pool(name="smallp", bufs=4))
        invbp = ctx.enter_context(tc.tile_pool(name="invbp", bufs=2))
        outp = ctx.enter_context(tc.tile_pool(name="outp", bufs=3))
        ps = ctx.enter_context(tc.tile_pool(name="ps", bufs=3, space="PSUM"))
        cxps = ctx.enter_context(tc.tile_pool(name="cxps", bufs=1, space="PSUM"))

        # ---------------- persistent inputs ----------------
        xt = const.tile([P, KCH, seq], bf16, name="xt_sb")
        wq = const.tile([P, KCH, DH], bf16, name="wq_sb")
        wk = const.tile([P, KCH, DH], bf16, name="wk_sb")
        wv = const.tile([P, KCH, DH], bf16, name="wv_sb")
        wo = const.tile([P, n_wchunks, DIN], bf16, name="wo_sb")
        wo2b = const.tile([64, DIN], bf16, name="wo2b_sb")
        bias = const.tile([P, DIN // P], f32, name="bias_sb")
        tri = const.tile([P, P], bf16, name="tri_sb")

        # DMA order: per d_in chunk, xt on the sync queue and wq/wk on the
        # scalar queue, so the kc-outer first projections can start as soon
        # as chunk 0 lands.
        for c in range(KCH):
            if c == 0:
                # split the first chunk so the first projection matmul can
                # start as soon as its 512-col slice lands
                q4 = seq // 4
                for qtr in range(4):
                    qs = slice(qtr * q4, (qtr + 1) * q4)
                    nc.sync.dma_start(xt[:, 0, qs], xt_d[0, :, qs])
            else:
                nc.sync.dma_start(xt[:, c, :], xt_d[c])
            nc.scalar.dma_start(wq[:, c, :], wq_d[c])
            nc.scalar.dma_start(wk[:, c, :], wk_d[c])
            nc.scalar.dma_start(wv[:, c, :], wv_d[c])
        nc.scalar.dma_start(tri[:], tri_d[:])
        for c in range(n_wchunks):
            nc.scalar.dma_start(wo[:, c, :], wo_d[c])
        nc.scalar.dma_start(wo2b[:], wo2b_d[:])
        nc.scalar.dma_start(bias[:], bias_d[:])

        # persistent activations
        qt = io.tile([P, HP, seq], bf16, name="qt_sb")
        ttl = io.tile([64, QW], bf16, name="ttl_sb")  # tail block's h1 ctx
        kt = io.tile([P, HP, seq], bf16, name="kt_sb")
        vx = io.tile([P, nkc, NH, 65], bf16, name="vx_sb")
        cn = io.tile([P, HP, seq], bf16, name="cn_sb")
        nc.vector.memset(vx[:, :, :, 64:65], 1.0)

        def qk0_upfront():
            """Pair-0 Q^T and K^T, kc-outer over 4 PSUM accumulators so each
            matmul only needs xt chunk kc (overlaps the input DMA stream)."""
            specs = []
            for wt, dst in ((wq, qt), (wk, kt)):
                for nb0 in (0, 2):
                    nbs = tuple(nb for nb in (nb0, nb0 + 1) if nb < nqb)
                    if nbs:
                        specs.append((wt, dst, nbs))
            accs = []
            for si in range(len(specs)):
                if si == len(specs) - 1:
                    acc = cxps.tile([P, 2, QW], f32, tag="cx", name=f"qk0acc{si}")
                else:
                    acc = ps.tile([P, 2, QW], f32, tag="quad", name=f"qk0acc{si}")
                accs.append(acc)
            for kc in range(KCH):
                for acc, (wt, dst, nbs) in zip(accs, specs):
                    for r, nb in enumerate(nbs):
                        nc.tensor.matmul(
                            acc[:, r, :],
                            lhsT=wt[:, kc, 0:P],
                            rhs=xt[:, kc, nb * QW : (nb + 1) * QW],
                            start=(kc == 0),
                            stop=(kc == KCH - 1),
                        )
            for acc, (wt, dst, nbs) in zip(accs, specs):
                for r, nb in enumerate(nbs):
                    nc.vector.tensor_copy(dst[:, 0, nb * QW : (nb + 1) * QW], acc[:, r, :])

        def qk_quarter(pair, quarter):
            """Project one quarter of pair's Q^T/K^T: one weight chunk reused
            across two 512-wide n blocks (kc-outer keeps LDWEIGHTS warm).
            Yields after each matmul so the caller can interleave."""
            wt, dst = (wq, qt) if quarter < 2 else (wk, kt)
            nbs = (0, 1) if quarter % 2 == 0 else (2, 3)
            if nbs[-1] >= nqb:  # small-seq (sim) builds
                nbs = tuple(nb for nb in nbs if nb < nqb)
                if not nbs:
                    return
            pt = ps.tile([P, 2, QW], f32, tag="quad", name="pt")
            for kc in range(KCH):
                for r, nb in enumerate(nbs):
                    nc.tensor.matmul(
                        pt[:, r, :],
                        lhsT=wt[:, kc, pair * P : (pair + 1) * P],
                        rhs=xt[:, kc, nb * QW : (nb + 1) * QW],
                        start=(kc == 0),
                        stop=(kc == KCH - 1),
                    )
                    yield
            for r, nb in enumerate(nbs):
                nc.vector.tensor_copy(dst[:, pair, nb * QW : (nb + 1) * QW], pt[:, r, :])

        def v_chunk(nb):
            """Yields after each matmul so the caller can interleave."""
            pt = ps.tile([P, 2, QW], f32, tag="quad", name="pt")
            for kc in range(KCH):
                nc.tensor.matmul(
                    pt[:, 0, :DH],
                    lhsT=xt[:, kc, nb * P : (nb + 1) * P],
                    rhs=wv[:, kc, :],
                    start=(kc == 0),
                    stop=(kc == KCH - 1),
                )
                yield
            nc.vector.tensor_copy(
                vx[:, nb, :, 0:64],
                pt[:, 0, :DH].rearrange("p (h d) -> p h d", d=64),
            )

        def out_proj_t(e0, qb, split_c2=False):
            """Transposed output projection for TWO e-chunks (both halves of
            one PSUM tile): out^T[e-chunk, q-block] = Wo_chunk^T @ cn, bias
            as a per-partition scalar.  c=0/1 for both e-chunks run first so
            only the c=2 matmuls (this core's last head pair) wait on the
            pair-2 normalization.  Yields after each matmul."""
            qsl = slice(qb * QW, (qb + 1) * QW)
            op = ps.tile([P, 2, QW], f32, tag="quad", name="op")
            es = [e for e in (e0, e0 + 1) if e < DIN // P]
            for c in range(n_wchunks - 1):
                for r, e in enumerate(es):
                    nc.tensor.matmul(
                        op[:, r, :],
                        lhsT=wo[:, c, e * P : (e + 1) * P],
                        rhs=cn[:, c, qsl],
                        start=(c == 0),
                        stop=False,
                    )
                    yield
            c2 = n_wchunks - 1
            for r, e in enumerate(es):
                if split_c2:
                    nc.tensor.matmul(
                        op[:, r, :],
                        lhsT=wo[0:64, c2, e * P : (e + 1) * P],
                        rhs=cn[0:64, c2, qsl],
                        start=False,
                        stop=False,
                    )
                    yield
                    nc.tensor.matmul(
                        op[:, r, :],
                        lhsT=wo2b[:, e * P : (e + 1) * P],
                        rhs=ttl[:],
                        start=False,
                        stop=True,
                    )
                else:
                    nc.tensor.matmul(
                        op[:, r, :],
                        lhsT=wo[:, c2, e * P : (e + 1) * P],
                        rhs=cn[:, c2, qsl],
                        start=False,
                        stop=True,
                    )
                yield
            ob = outp.tile([P, 2, QW], f32, name="ob")
            for r, e in enumerate(es):
                nc.vector.tensor_scalar_add(ob[:, r, :], op[:, r, :], bias[:, e : e + 1])
            if len(es) == 2:
                # one DMA for both e-chunks: dst AP reordered to match the
                # (partition, e, q) source layout
                nc.sync.dma_start(
                    out_d[es[0] : es[0] + 2, :, qsl].rearrange("e p q -> p e q"),
                    ob[:],
                )
            else:
                nc.sync.dma_start(out_d[es[0], :, qsl], ob[:, 0, :])

        def finisher(pair, i, cr, tail=0, last=False):
            """Deferred per-block softmax normalization, run as fill work.
            The denominator row lives on partition 64 and engines can't
            cross partitions, so: DMA-scatter it across 128 lanes,
            reciprocal, DRAM round-trip partition-broadcast, per-q scale on
            DVE, head-1 partition move via SBUF->SBUF DMA.  The yields pace
            each op's queue position to just after its data dependency;
            `tail` delays whatever is queued next (out_proj) past the DMA
            chain's latency."""
            qsl = slice(i * QW, (i + 1) * QW)
            # move the denominator row to partition 0 (engines can't cross
            # partitions; gpsimd's broadcast only reads partition 0)
            dn = smallp.tile([1, 2, QW], f32, tag="dn", bufs=2, name="dn")
            nc.sync.dma_start(dn[:], cr[64:65, :, :])
            yield
            yield
            db = invbp.tile([64, 2, QW], f32, tag="db", name="db")
            nc.gpsimd.partition_broadcast(db[:], dn[0:1, :, :])
            yield
            ib = invbp.tile([64, 2, QW], f32, tag="ib", name="ib")
            nc.vector.reciprocal_approx_fast(ib[:], db[:])
            yield
            nc.vector.tensor_mul(cn[0:64, pair, qsl], cr[0:64, 0, :], ib[:, 0, :])
            yield
            if last:
                # h1 stays in ttl (partitions 0-63); the split c2b matmul
                # reads it directly, skipping the partition-move DMA
                nc.vector.tensor_mul(ttl[:], cr[0:64, 1, :], ib[:, 1, :])
            else:
                tt = smallp.tile([64, QW], bf16, tag="tt", name="tt")
                nc.vector.tensor_mul(tt[:], cr[0:64, 1, :], ib[:, 1, :])
                yield
                nc.sync.dma_start(cn[64:128, pair, qsl], tt[:])
            for _ in range(tail):
                yield

        class FillQueue:
            """Queue of instruction generators, driven a few matmuls at a
            time from inside the attention loop to fill PE bubbles.
            Finishers are urgent and small: they jump the queue, but never
            interrupt a generator that has already started (its PSUM
            accumulator is live)."""

            def __init__(self):
                self.gens = []  # (label, gen)
                self.head_started = False

            def add(self, gen, label=None):
                self.gens.append((label, gen))

            def add_front(self, gen, label=None):
                pos = 1 if (self.head_started and self.gens) else 0
                self.gens.insert(pos, (label, gen))

            def _advance(self):
                """One step of the head generator. Returns False if empty."""
                if not self.gens:
                    return False
                try:
                    next(self.gens[0][1])
                    self.head_started = True
                except StopIteration:
                    self.gens.pop(0)
                    self.head_started = False
                return True

            def step(self, n):
                while n > 0 and self.gens:
                    self._advance()
                    n -= 1

            def drain_matching(self, label):
                """Run all generators with this label to completion, in order,
                leaving other generators queued (and positioned)."""
                k = 0
                while k < len(self.gens):
                    if self.gens[k][0] == label:
                        gen = self.gens[k][1]
                        while True:
                            try:
                                next(gen)
                            except StopIteration:
                                break
                        self.gens.pop(k)
                        if k == 0:
                            self.head_started = False
                    else:
                        k += 1

            def count(self, label):
                return sum(1 for lab, _ in self.gens if lab == label)

            def drain(self):
                while self.gens:
                    self._advance()

        def attention_qblock(pair, i, fq, steps=(2, 3)):
            """One 512-wide q block of causal attention for a head pair.
            fq: FillQueue driven mid-loop to fill PE bubbles."""
            nj = 4 * i + 4  # active k chunks
            cx = cxps.tile([65, 2, QW], f32, tag="cx", name="cx")
            pend = []  # (j, exp tile) awaiting ctx matmuls

            def ctx_mms(j, et):
                d = j - 4 * i
                off = P * d if d > 0 else 0
                for hh in (0, 1):
                    nc.tensor.matmul(
                        cx[:, hh, off:],
                        lhsT=vx[:, j, 2 * pair + hh, :],
                        rhs=et[:, hh, off:],
                        start=(j == 0),
                        stop=(j == nj - 1),
                    )

            for j in range(nj):
                d = j - 4 * i
                off = P * d if d > 0 else 0
                sc = ps.tile([P, 2, QW], f32, tag="quad", name="sc")
                for hh in (0, 1):
                    base = 64 * hh
                    nc.tensor.matmul(
                        sc[:, hh, off:],
                        lhsT=kt[base : base + 64, pair, j * P : (j + 1) * P],
                        rhs=qt[base : base + 64, pair, i * QW + off : (i + 1) * QW],
                        start=True,
                        stop=True,
                        tile_position=(base, 0),
                    )
                et = expp.tile([P, 2, QW], bf16, name="et")
                if off:
                    nc.scalar.activation(
                        et[:, :, off:], sc[:, :, off:], EXP, scale=0.125
                    )
                else:
                    nc.scalar.activation(et[:], sc[:], EXP, scale=0.125)
                if d >= 0:
                    for hh in (0, 1):
                        nc.vector.tensor_mul(
                            et[:, hh, off : off + P], et[:, hh, off : off + P], tri[:]
                        )
                if len(pend) >= 2:
                    ctx_mms(*pend.pop(0))
                pend.append((j, et))
                fq.step(steps[1] if d >= 0 else steps[0])
            for args in pend:
                ctx_mms(*args)

            # ctx + denominators out of PSUM; normalization deferred to a
            # finisher so the next block's matmuls proceed immediately.
            # Safety valve: bound pending finishers (each pins a cr buffer)
            # by draining from the front, preserving FIFO start order.
            while fq.count("fin") > 2:
                fq._advance()
            cr = crawp.tile([65, 2, QW], f32, name="cr")
            if pair == HP - 1 and i == nqb - 1:
                # very last block: copy the denominator row first so the
                # finisher's DMA chain starts ~0.5us earlier
                nc.vector.tensor_copy(cr[64:65, :, :], cx[64:65, :, :])
                nc.vector.tensor_copy(cr[0:64, :, :], cx[0:64, :, :])
            else:
                nc.vector.tensor_copy(cr[:], cx[:])
            fq.add_front(
                finisher(
                    pair,
                    i,
                    cr,
                    tail=8 if pair == HP - 1 else 0,
                    last=False,
                ),
                "fin",
            )

        # ---------------- schedule ----------------
        BISECT_QK0 = False
        if BISECT_QK0:
            for quarter in range(4):
                for _ in qk_quarter(0, quarter):
                    pass
        else:
            qk0_upfront()
        for nb in range(nkc):
            for _ in v_chunk(nb):
                pass
        fq = FillQueue()
        for pair in range(HP):
            last_pair = pair == HP - 1
            if not last_pair:
                for quarter in range(4):
                    fq.add(qk_quarter(pair + 1, quarter), f"qk{pair + 1}")
            for i in range(nqb):
                attention_qblock(pair, i, fq, steps=(3, 4))
                if last_pair:
                    for e0 in range(0, DIN // P, 2):
                        fq.add(out_proj_t(e0, i, split_c2=False), "op")
            if not last_pair:
                fq.drain_matching(f"qk{pair + 1}")
        fq.drain()

    nc.compile()
    return nc


def make_in_maps(X, Wq, Wk, Wv, Wo, bo, seq=2048):
    """Shard full inputs into the 8 per-core input maps."""
    X = np.asarray(X, np.float32)
    Wq = np.asarray(Wq, np.float32)
    Wk = np.asarray(Wk, np.float32)
    Wv = np.asarray(Wv, np.float32)
    Wo = np.asarray(Wo, np.float32)
    bo = np.asarray(bo, np.float32)

    tri = np.triu(np.ones((P, P), np.float32)).astype(BF16)
    bias_full = np.ascontiguousarray(bo.reshape(DIN // P, P).T).astype(np.float32)
    bias_zero = np.zeros((P, DIN // P), np.float32)

    in_maps = []
    for b in range(X.shape[0]):
        xt = np.ascontiguousarray(X[b].T).astype(BF16).reshape(KCH, P, seq)
        for hg in range(2):
            sl = slice(hg * DH, (hg + 1) * DH)
            in_maps.append(
                {
                    "xt": xt,
                    "wq": np.ascontiguousarray(Wq[:, sl]).astype(BF16).reshape(KCH, P, DH),
                    "wk": np.ascontiguousarray(Wk[:, sl]).astype(BF16).reshape(KCH, P, DH),
                    "wv": np.ascontiguousarray(Wv[:, sl]).astype(BF16).reshape(KCH, P, DH),
                    "wo": np.ascontiguousarray(Wo[sl, :]).astype(BF16).reshape(3, P, DIN),
                    "wo2b": np.ascontiguousarray(Wo[sl, :][320:384, :]).astype(BF16),
                    "bias": bias_full if hg == 0 else bias_zero,
                    "tri": tri,
                }
            )
    return in_maps


_built = None


def _get_built():
    global _built
    if _built is None:
        _built = build()
    return _built


def run(inputs, trace=False):
    from concourse.bass_utils import run_bass_kernel_spmd

    nc = _get_built()
    in_maps = make_in_maps(**inputs)
    res = run_bass_kernel_spmd(nc, in_maps, list(range(8)), trace=trace)
    # per-core output is stored transposed as [6, 128, seq] = out.T chunked
    parts = [
        np.asarray(r["out"], np.float32).reshape(DIN, -1).T for r in res.results
    ]
    out = np.stack([parts[2 * b] + parts[2 * b + 1] for b in range(len(parts) // 2)])
    return out, res


def kernel(X, Wq, Wk, Wv, Wo, bo):
    out, _ = run(dict(X=X, Wq=Wq, Wk=Wk, Wv=Wv, Wo=Wo, bo=bo))
    return out
